# revision 5
# baseline (speedup 1.0000x reference)
"""Trainium2 Bass kernel for nn_MLFF_dmirror (dense MLP force field).

Math (per batch b):
  z1 = x@W1+b1; a1 = softplus(z1); z2 = a1@W2+b2; a2 = softplus(z2)
  Ei = a2@W3+b3;  Etot[b] = sum_i Ei
  g  = dEi/dx = ((W3*sig(z2))@W2.T*sig(z1))@W1.T          # [N, F]
  Force[b,i,c] = sum_{j,f} g[j,f] * dfeat[b,i,j,f,c]

Sharding: 8 cores; core k handles batch b=k//2, atom half h=k%2.
Each core redundantly runs the (tiny) per-batch MLP fwd+bwd for all 256
atoms, then contracts its local 25.2MB dfeat shard [128, 16384, 3]
against g.  No collectives needed.

Device layout for the big contraction (HBM-bound, ~70us/core roofline):
  dfeat row i -> tile t[p=jf_hi(128), q=jf_lo(128), c(3)]   (contig 1536B/part)
  g12[p, q*3+c] = g[jf=128p+q]  (g replicated x3 over c, x4 over atom group)
  DVE:  m = t * g12  (bf16 out)
  PE :  R[i, (q,c)] += sum_p m[p, (q,c)]   via ones-column-at-i lhsT window
  DVE:  Force[i, c] = sum_q R[i, q*3+c]    (3 strided reduces)
"""

import os
import sys

for _p in ("/opt/trn_rl_repo",):
    if os.path.isdir(_p) and _p not in sys.path:
        sys.path.insert(0, _p)

import numpy as np

import concourse.bass as bass
import concourse.bacc as bacc
import concourse.tile as tile
from concourse import mybir
from concourse import bass_utils

B, N, F = 4, 256, 64
H1, H2 = 128, 64
NLOC = N // 2          # atoms per core
JF = N * F             # 16384
P = 128                # jf_hi
Q = JF // P            # 128 jf_lo
GRP = 4                # atoms per DVE tile
FP32 = mybir.dt.float32
BF16 = mybir.dt.bfloat16

_CACHE = {}


def build_nc():
    nc = bacc.Bacc()

    d_imT = nc.dram_tensor("imageT", [F, N], FP32, kind="ExternalInput")
    d_df = nc.dram_tensor("dfeat", [NLOC, JF * 3], FP32, kind="ExternalInput")
    d_W1 = nc.dram_tensor("W1", [F, H1], FP32, kind="ExternalInput")
    d_W1T = nc.dram_tensor("W1T", [H1, F], FP32, kind="ExternalInput")
    d_W2 = nc.dram_tensor("W2", [H1, H2], FP32, kind="ExternalInput")
    d_W2T = nc.dram_tensor("W2T", [H2, H1], FP32, kind="ExternalInput")
    d_W3 = nc.dram_tensor("W3", [H2, 1], FP32, kind="ExternalInput")
    d_b1 = nc.dram_tensor("b1c", [H1, 1], FP32, kind="ExternalInput")
    d_b2 = nc.dram_tensor("b2c", [H2, 1], FP32, kind="ExternalInput")
    d_b3 = nc.dram_tensor("b3c", [1, 1], FP32, kind="ExternalInput")
    d_force = nc.dram_tensor("force", [NLOC, 3], FP32, kind="ExternalOutput")
    d_etot = nc.dram_tensor("etot", [1, 1], FP32, kind="ExternalOutput")

    AF = mybir.ActivationFunctionType
    AX = mybir.AxisListType

    with tile.TileContext(nc) as tc:
        with (
            tc.tile_pool(name="singles", bufs=1) as singles,
            tc.tile_pool(name="acts", bufs=1) as acts,
            tc.tile_pool(name="dstream", bufs=4) as dstream,
            tc.tile_pool(name="mstream", bufs=4) as mstream,
            tc.tile_pool(name="psA", bufs=1, space="PSUM") as psA,
            tc.tile_pool(name="psR", bufs=1, space="PSUM") as psR,
        ):
            # ---- constant / weight loads -------------------------------
            imT = singles.tile([F, N], FP32)
            nc.sync.dma_start(out=imT, in_=d_imT[:, :])
            W1 = singles.tile([F, H1], FP32)
            nc.sync.dma_start(out=W1, in_=d_W1[:, :])
            W1T = singles.tile([H1, F], FP32)
            nc.sync.dma_start(out=W1T, in_=d_W1T[:, :])
            W2 = singles.tile([H1, H2], FP32)
            nc.sync.dma_start(out=W2, in_=d_W2[:, :])
            W2T = singles.tile([H2, H1], FP32)
            nc.sync.dma_start(out=W2T, in_=d_W2T[:, :])
            W3 = singles.tile([H2, 1], FP32)
            nc.sync.dma_start(out=W3, in_=d_W3[:, :])
            b1 = singles.tile([H1, 1], FP32)
            nc.sync.dma_start(out=b1, in_=d_b1[:, :])
            b2 = singles.tile([H2, 1], FP32)
            nc.sync.dma_start(out=b2, in_=d_b2[:, :])
            b3 = singles.tile([1, 1], FP32)
            nc.sync.dma_start(out=b3, in_=d_b3[:, :])

            # ones-column window for row-select matmuls: onesw[:, 128+m]==1 iff m==0
            onesw = singles.tile([P, 2 * P], BF16)
            nc.vector.memset(onesw, 0.0)
            nc.vector.memset(onesw[:, P : P + 1], 1.0)

            # ---- Phase A: MLP fwd + bwd for all 256 atoms of this batch -
            # Only exp+ln live in one ACT LUT set, so build softplus and
            # sigmoid from them: e=exp(z+b); sp=ln(1+e); sig=e/(1+e).
            def softplus_sigmoid(z, bias, h):
                e = acts.tile([h, N], FP32, tag=f"e{h}")
                nc.scalar.activation(e, z, AF.Exp, bias=bias)
                p1 = acts.tile([h, N], FP32, tag=f"p1{h}")
                nc.vector.tensor_scalar_add(p1, e, 1.0)
                sp = acts.tile([h, N], FP32, tag=f"sp{h}")
                nc.scalar.activation(sp, p1, AF.Ln)
                r = acts.tile([h, N], FP32, tag=f"r{h}")
                nc.vector.reciprocal(r, p1)
                sg = acts.tile([h, N], FP32, tag=f"sg{h}")
                nc.vector.tensor_mul(sg, e, r)
                return sp, sg

            z1 = psA.tile([H1, N], FP32)
            nc.tensor.matmul(z1, W1, imT, start=True, stop=True)
            a1, sg1 = softplus_sigmoid(z1, b1, H1)

            z2 = psA.tile([H2, N], FP32)
            nc.tensor.matmul(z2, W2, a1, start=True, stop=True)
            a2, sg2 = softplus_sigmoid(z2, b2, H2)

            Ep = psA.tile([1, N], FP32)
            nc.tensor.matmul(Ep, W3, a2, start=True, stop=True)
            Ei = acts.tile([1, N], FP32)
            nc.vector.tensor_scalar_add(Ei, Ep, b3)
            etot = acts.tile([1, 1], FP32)
            nc.vector.reduce_sum(out=etot, in_=Ei, axis=AX.X)
            nc.sync.dma_start(out=d_etot[:, :], in_=etot)

            dz2 = acts.tile([H2, N], FP32)
            nc.vector.tensor_scalar_mul(dz2, sg2, W3)
            da1 = psA.tile([H1, N], FP32)
            nc.tensor.matmul(da1, W2T, dz2, start=True, stop=True)
            dz1 = acts.tile([H1, N], FP32)
            nc.vector.tensor_mul(dz1, da1, sg1)

            # g_nat[j, f] for j=2p+r packed as g2[p, r*64+f]
            g2p = psA.tile([P, Q], FP32)
            dz1v = dz1.rearrange("h (p r) -> h r p", r=2)
            for r in range(2):
                nc.tensor.matmul(
                    g2p[:, r * F : (r + 1) * F], dz1v[:, r, :], W1T,
                    start=True, stop=True,
                )

            # g12[p, (i2, q, c)] = g2[p, q]  (x3 over c, x4 over atom group)
            g12 = singles.tile([P, GRP, Q, 3], FP32)
            for c in range(3):
                nc.vector.tensor_copy(g12[:, 0, :, c], g2p)
            nc.vector.tensor_copy(g12[:, 1, :, :], g12[:, 0, :, :])
            nc.vector.tensor_copy(
                g12.rearrange("p i q c -> p (i q c)")[:, 2 * Q * 3 : 4 * Q * 3],
                g12.rearrange("p i q c -> p (i q c)")[:, 0 : 2 * Q * 3],
            )

            # ---- Phase B: stream dfeat, g-multiply, row-reduce on PE ----
            R = psR.tile([P, Q * 3], FP32)
            dfv = d_df.rearrange("i (p m) -> i p m", p=P)  # m = q*3+c
            n_grp = NLOC // GRP
            for k in range(n_grp):
                t4 = dstream.tile([P, GRP, Q * 3], FP32)
                for a in range(GRP):
                    nc.sync.dma_start(
                        out=t4[:, a, :], in_=dfv[GRP * k + a, :, :]
                    )
                m4 = mstream.tile([P, GRP, Q * 3], BF16)
                nc.vector.tensor_mul(
                    m4, t4, g12.rearrange("p i q c -> p i (q c)")
                )
                for a in range(GRP):
                    i = GRP * k + a
                    nc.tensor.matmul(
                        R, onesw[:, P - i : 2 * P - i], m4[:, a, :],
                        start=(i == 0), stop=(i == NLOC - 1),
                    )

            # ---- Phase C: Force[i, c] = sum_q R[i, q*3+c] ---------------
            Fsb = acts.tile([NLOC, 3], FP32)
            Rv = R.rearrange("p (q c) -> p c q", c=3)
            for c in range(3):
                nc.vector.reduce_sum(
                    out=Fsb[:, c : c + 1], in_=Rv[:, c, :], axis=AX.X
                )
            nc.sync.dma_start(out=d_force[:, :], in_=Fsb)

    if not nc.is_finalized():
        nc.finalize()
    return nc


def prepare_in_maps(image, dfeat, W1, b1, W2, b2, W3, b3):
    f32 = np.float32
    W1 = np.ascontiguousarray(W1, f32)
    W2 = np.ascontiguousarray(W2, f32)
    W3 = np.ascontiguousarray(W3, f32).reshape(H2, 1)
    shared = {
        "W1": W1,
        "W1T": np.ascontiguousarray(W1.T),
        "W2": W2,
        "W2T": np.ascontiguousarray(W2.T),
        "W3": W3,
        "b1c": np.ascontiguousarray(b1, f32).reshape(H1, 1),
        "b2c": np.ascontiguousarray(b2, f32).reshape(H2, 1),
        "b3c": np.ascontiguousarray(b3, f32).reshape(1, 1),
    }
    in_maps = []
    for k in range(8):
        b, h = k // 2, k % 2
        in_maps.append(
            {
                "imageT": np.ascontiguousarray(np.asarray(image[b], f32).T),
                "dfeat": np.ascontiguousarray(
                    np.asarray(dfeat[b, h * NLOC : (h + 1) * NLOC], f32)
                ).reshape(NLOC, JF * 3),
                **shared,
            }
        )
    return in_maps


def kernel(image, dfeat, neighbor=None, Egroup_weight=None, divider=None,
           W1=None, b1=None, W2=None, b2=None, W3=None, b3=None):
    if "nc" not in _CACHE:
        _CACHE["nc"] = build_nc()
    nc = _CACHE["nc"]
    in_maps = prepare_in_maps(image, dfeat, W1, b1, W2, b2, W3, b3)
    res = bass_utils.run_bass_kernel_spmd(nc, in_maps, core_ids=list(range(8)))
    outs = res.results if hasattr(res, "results") else res
    Etot = np.zeros((B,), np.float32)
    Force = np.zeros((B, N, 3), np.float32)
    for k in range(8):
        b, h = k // 2, k % 2
        Force[b, h * NLOC : (h + 1) * NLOC] = outs[k]["force"]
        if h == 0:
            Etot[b] = outs[k]["etot"][0, 0]
    return Etot, Force
